# revision 13
# baseline (speedup 1.0000x reference)
"""GCNDecoder on 8 Trainium2 NeuronCores (Bass/Tile).

3-layer GCN: (GCNConv -> BN -> ReLU) x2 -> GCNConv, N=50000 nodes, E=800000
edges, feature dims 256 -> 512 -> 1024 -> 3000.

Strategy (data-parallel over nodes, per the sharding hint):
  * Reassociate each layer as (A_hat @ h) @ W: aggregate FIRST in the smaller
    input-feature dim, then dense-matmul.  A_hat includes self loops.
  * Nodes are permuted and padded to 53248 = 8 cores x 52 blocks x 128 so
    every core owns an equal shard; a greedy 2-D bin-pack equalizes per-block
    in-edge counts, so the SPMD program is identical on all cores.
  * Edge aggregation: gather h[src] rows with the GPSIMD dma_gather ucode
    (int16 indices -> the node table is split by slot parity into two
    strided views of <=26624 rows each), then segment-sum via TensorEngine
    matmuls against host-precomputed one-hot*coef matrices (S).
  * Dense matmul produces the TRANSPOSED activation [F_out, nodes] so that
    BatchNorm stats (free-dim reduce + per-partition affine) are native; the
    scalar engine's accum_out gives row sums for free.
  * BN statistics: AllReduce of per-core (sum, sumsq); the BN beta/bias of
    the reference cancels in BN so layer biases b1, b2 are dropped; b3 is
    added on the host.  BN renormalizes, relu, transpose back to row-major,
    then AllGather re-replicates the node table for the next layer's gather.
  * Everything on-device is bf16 with fp32 PSUM accumulation (rel-err budget
    2e-2).
"""

import os
import time

import numpy as np
import ml_dtypes

BF16 = ml_dtypes.bfloat16

# ---------------------------------------------------------------- config

REAL_CFG = dict(
    N=50000, F_IN=256, H1=512, H2=1024, OUT=3000, OUT_PAD=3072,
    NCORES=8, BLOCKS_PER_CORE=52, GROUP_BLOCKS=4, EPS=1e-5,
)


def _derived(cfg):
    cfg = dict(cfg)
    cfg["NP"] = cfg["NCORES"] * cfg["BLOCKS_PER_CORE"] * 128
    cfg["NGRP"] = cfg["BLOCKS_PER_CORE"] // cfg["GROUP_BLOCKS"]
    assert cfg["BLOCKS_PER_CORE"] % cfg["GROUP_BLOCKS"] == 0
    return cfg


# ---------------------------------------------------------------- host prep

def _preprocess(x, edge_index, edge_attr, cfg):
    """Permute/pad nodes, classify+pack edges, build S / index tables."""
    N, NP = cfg["N"], cfg["NP"]
    nb_tot = NP // 128

    src = np.asarray(edge_index[0], dtype=np.int64)
    dst = np.asarray(edge_index[1], dtype=np.int64)
    w = np.asarray(edge_attr, dtype=np.float32)[:, 2]

    deg = np.bincount(dst, weights=w, minlength=N).astype(np.float32) + 1.0
    dinv = 1.0 / np.sqrt(deg)
    coef = (dinv[src] * w * dinv[dst]).astype(np.float32)
    selfc = (dinv * dinv).astype(np.float32)

    # ---- parity assignment: balance total out-degree between parities
    out_deg = np.bincount(src, minlength=N)
    order = np.argsort(-out_deg, kind="stable")
    parity = np.empty(N, np.int8)
    parity[order] = (np.arange(N) % 2).astype(np.int8)

    # per-node in-degree counts by src parity (slot counts, unweighted)
    a_deg = np.bincount(dst[parity[src] == 0], minlength=N)
    b_deg = np.bincount(dst[parity[src] == 1], minlength=N)

    # ---- greedy 2-D bin pack: nodes -> blocks (64 even + 64 odd slots each)
    loadA = np.zeros(nb_tot, np.float64)
    loadB = np.zeros(nb_tot, np.float64)
    quota = np.zeros((nb_tot, 2), np.int32)  # filled count per parity
    blk_of_node = np.empty(N, np.int32)
    nodes_by_sz = np.argsort(-(a_deg + b_deg), kind="stable")
    INF = 1e18
    for v in nodes_by_sz:
        p = parity[v]
        da = a_deg[v] + (1 - p)  # own self-edge contributes to its parity class
        db = b_deg[v] + p
        cost = np.maximum(loadA + da, loadB + db)
        cost[quota[:, p] >= 64] = INF
        b = int(np.argmin(cost))
        blk_of_node[v] = b
        loadA[b] += da
        loadB[b] += db
        quota[b, p] += 1

    # ---- slots: even-parity nodes at even slots, odd at odd
    slot_of_node = np.empty(N, np.int64)
    fill = np.zeros((nb_tot, 2), np.int64)
    # stable order within block: iterate nodes grouped by (block, parity)
    order2 = np.lexsort((parity, blk_of_node))
    for v in order2:
        b = blk_of_node[v]
        p = parity[v]
        slot_of_node[v] = b * 128 + 2 * fill[b, p] + p
        fill[b, p] += 1

    # ---- edge arrays (real edges + self edges)
    e_src_slot = slot_of_node[src]
    e_dst_slot = slot_of_node[dst]
    all_src = np.concatenate([e_src_slot, slot_of_node])
    all_dst = np.concatenate([e_dst_slot, slot_of_node])
    all_coef = np.concatenate([coef, selfc])

    e_cls = (all_src % 2).astype(np.int8)  # 0 = table A (even), 1 = B
    e_tab = (all_src // 2).astype(np.int64)
    e_blk = (all_dst // 128).astype(np.int64)
    e_dloc = (all_dst % 128).astype(np.int64)

    # rank within (block, class)
    sidx = np.lexsort((e_cls, e_blk))
    sb, sc = e_blk[sidx], e_cls[sidx]
    key = sb * 2 + sc
    _, starts, counts = np.unique(key, return_index=True, return_counts=True)
    rank = np.arange(len(sidx)) - np.repeat(starts, counts)
    nA = np.zeros(nb_tot, np.int64)
    nB = np.zeros(nb_tot, np.int64)
    kk = np.zeros(nb_tot * 2, np.int64)
    kk[np.unique(key)] = counts
    nA = kk[0::2]
    nB = kk[1::2]
    K_A = int(np.ceil(nA.max() / 128))
    K_B = int(np.ceil(nB.max() / 128))
    KT = K_A + K_B

    # ---- per-core packed structures
    nblk = cfg["BLOCKS_PER_CORE"]
    ncores = cfg["NCORES"]
    S_cores, idxA_cores, idxB_cores = [], [], []
    bl_l = (sb % nblk).astype(np.int64)
    core_l = (sb // nblk).astype(np.int64)
    chunk = np.where(sc == 0, rank // 128, K_A + rank // 128)
    part = rank % 128
    s_coef = all_coef[sidx].astype(np.float32)
    s_dloc = e_dloc[sidx]
    s_tab = e_tab[sidx]
    for c in range(ncores):
        m = core_l == c
        S = np.zeros((nblk, 128, KT * 128), np.float32)
        S[bl_l[m], part[m], chunk[m] * 128 + s_dloc[m]] = s_coef[m]
        S_cores.append(S.astype(BF16))

        idxA = np.zeros((nblk, K_A * 128), np.int16)
        idxB = np.zeros((nblk, K_B * 128), np.int16)
        ma = m & (sc == 0)
        mb = m & (sc == 1)
        idxA[bl_l[ma], rank[ma]] = s_tab[ma].astype(np.int16)
        idxB[bl_l[mb], rank[mb]] = s_tab[mb].astype(np.int16)

        def pack(idx, K):
            # slot i of each block -> [i % 16, i // 16], replicated x8
            t = idx.reshape(nblk, K * 8, 16).transpose(2, 0, 1).reshape(16, -1)
            return np.tile(t, (8, 1)).copy()

        idxA_cores.append(pack(idxA, K_A))
        idxB_cores.append(pack(idxB, K_B))

    # ---- node table for layer 1
    x = np.asarray(x, dtype=np.float32)
    x_tab = np.zeros((NP, cfg["F_IN"]), BF16)
    x_tab[slot_of_node] = x.astype(BF16)

    return dict(
        K_A=K_A, K_B=K_B, KT=KT, slot_of_node=slot_of_node,
        S_cores=S_cores, idxA_cores=idxA_cores, idxB_cores=idxB_cores,
        x_tab=x_tab,
    )


def _feat_t(v, F):
    """[F] per-feature vector -> [128, F//128] tile layout (f at [f%128, f//128])."""
    v = np.asarray(v, dtype=np.float32)
    out = np.zeros((128, F // 128), np.float32)
    out[:, :] = v.reshape(F // 128, 128).T
    return out


# ---------------------------------------------------------------- builder

def _build(cfg, K_A, K_B):
    import concourse.bass as bass
    import concourse.bacc as bacc
    import concourse.mybir as mybir
    import concourse.tile as tile
    from concourse.masks import make_identity

    NP = cfg["NP"]
    F_IN, H1, H2, OUT_PAD = cfg["F_IN"], cfg["H1"], cfg["H2"], cfg["OUT_PAD"]
    nblk, GB, ngrp = cfg["BLOCKS_PER_CORE"], cfg["GROUP_BLOCKS"], cfg["NGRP"]
    ncores = cfg["NCORES"]
    KT = K_A + K_B
    GW = GB * 128  # node-group width
    INV_N = 1.0 / cfg["N"]
    EPS = cfg["EPS"]
    groups = [list(range(ncores))]
    f32 = mybir.dt.float32
    bf16 = mybir.dt.bfloat16

    nc = bacc.Bacc("TRN2", target_bir_lowering=False, debug=False,
                   num_devices=ncores)

    x_tab = nc.dram_tensor("x_tab", [NP, F_IN], bf16, kind="ExternalInput")
    idxA_d = nc.dram_tensor("idxA", [128, nblk * K_A * 8], mybir.dt.int16,
                            kind="ExternalInput")
    idxB_d = nc.dram_tensor("idxB", [128, nblk * K_B * 8], mybir.dt.int16,
                            kind="ExternalInput")
    S_d = nc.dram_tensor("S", [nblk, 128, KT * 128], bf16, kind="ExternalInput")
    W1_d = nc.dram_tensor("W1", [F_IN, H1], bf16, kind="ExternalInput")
    W2_d = nc.dram_tensor("W2", [H1, H2], bf16, kind="ExternalInput")
    W3_d = nc.dram_tensor("W3", [H2, OUT_PAD], bf16, kind="ExternalInput")
    gb1_d = nc.dram_tensor("gb1", [128, 2 * (H1 // 128)], f32,
                           kind="ExternalInput")
    gb2_d = nc.dram_tensor("gb2", [128, 2 * (H2 // 128)], f32,
                           kind="ExternalInput")
    out_d = nc.dram_tensor("out", [nblk * 128, OUT_PAD], bf16,
                           kind="ExternalOutput")

    with tile.TileContext(nc) as tc:
        with (
            tc.tile_pool(name="const", bufs=1) as cpool,
            tc.tile_pool(name="work", bufs=2) as wpool,
            tc.tile_pool(name="psum", bufs=2, space="PSUM") as ppool,
            tc.tile_pool(name="dram", bufs=1, space="DRAM") as dpool,
        ):
            # ------- resident tiles
            ident = cpool.tile([128, 128], bf16, name="ident")
            make_identity(nc, ident[:])
            idxA_sb = cpool.tile([128, nblk * K_A * 8], mybir.dt.int16,
                                 name="idxA_sb")
            nc.sync.dma_start(idxA_sb[:], idxA_d[:])
            idxB_sb = cpool.tile([128, nblk * K_B * 8], mybir.dt.int16,
                                 name="idxB_sb")
            nc.sync.dma_start(idxB_sb[:], idxB_d[:])

            def load_w(wd, fin, fout, name):
                tiles = []
                for k2 in range(fin // 128):
                    t = cpool.tile([128, fout], bf16, name=f"{name}_{k2}")
                    nc.sync.dma_start(t[:], wd[k2 * 128:(k2 + 1) * 128, :])
                    tiles.append(t)
                return tiles

            W1_sb = load_w(W1_d, F_IN, H1, "w1")
            W2_sb = load_w(W2_d, H1, H2, "w2")
            W3_sb = load_w(W3_d, H2, OUT_PAD, "w3")
            gb1_sb = cpool.tile([128, 2 * (H1 // 128)], f32, name="gb1_sb")
            nc.sync.dma_start(gb1_sb[:], gb1_d[:])
            gb2_sb = cpool.tile([128, 2 * (H2 // 128)], f32, name="gb2_sb")
            nc.sync.dma_start(gb2_sb[:], gb2_d[:])

            # ------- internal DRAM
            pre1 = dpool.tile([H1, nblk * 128], bf16, name="pre1")
            pre2 = dpool.tile([H2, nblk * 128], bf16, name="pre2")
            ag1_in = dpool.tile([nblk * 128, H1], bf16, name="ag1_in")
            ag2_in = dpool.tile([nblk * 128, H2], bf16, name="ag2_in")
            _shared = ("Local" if os.environ.get("GCN_DBG_NOCOLL", "0") == "1"
                       else "Shared")
            h1 = dpool.tile([NP, H1], bf16, name="h1", addr_space=_shared)
            h2 = dpool.tile([NP, H2], bf16, name="h2", addr_space=_shared)
            ar1_in = dpool.tile([128, 2 * (H1 // 128)], f32, name="ar1_in")
            ar1_out = dpool.tile([128, 2 * (H1 // 128)], f32, name="ar1_out",
                                 addr_space="Shared")
            ar2_in = dpool.tile([128, 2 * (H2 // 128)], f32, name="ar2_in")
            ar2_out = dpool.tile([128, 2 * (H2 // 128)], f32, name="ar2_out",
                                 addr_space="Shared")

            Copy = mybir.ActivationFunctionType.Copy
            Square = mybir.ActivationFunctionType.Square
            Relu = mybir.ActivationFunctionType.Relu
            Sqrt = mybir.ActivationFunctionType.Sqrt

            def layer(h_src, F_in, F_out, W_sb, pre, ag_in, h_next,
                      ar_in, ar_out, gb_sb, lname, is_last=False):
                FC_in = F_in // 128
                FC_out = F_out // 128
                nch = (F_in + 511) // 512  # agg matmul N-chunks

                if os.environ.get("GCN_DBG_NOSTRIDE", "0") == "1":
                    viewA = h_src[0:NP // 2, :]
                    viewB = h_src[NP // 2:, :]
                    estep = F_in
                else:
                    hv = h_src[:].rearrange("(n two) f -> two n f", two=2)
                    viewA, viewB = hv[0], hv[1]
                    estep = 2 * F_in

                if not is_last:
                    sum_acc = cpool.tile([128, FC_out * ngrp], f32,
                                         name=f"{lname}_sum")
                    sq_acc = cpool.tile([128, FC_out * ngrp], f32,
                                        name=f"{lname}_sq")

                for g in range(ngrp):
                    gT = wpool.tile([128, FC_in * GW], bf16, tag="gT",
                                    name=f"{lname}_gT_{g}", bufs=2)
                    for j in range(GB):
                        b = GB * g + j
                        S_t = wpool.tile([128, KT * 128], bf16, tag="S_t",
                                         name=f"{lname}_S_{b}", bufs=2)
                        nc.sync.dma_start(S_t[:], S_d[b])
                        GA = wpool.tile([128, K_A * F_in], bf16, tag="GA",
                                        name=f"{lname}_GA_{b}", bufs=2)
                        GBt = wpool.tile([128, K_B * F_in], bf16, tag="GB",
                                         name=f"{lname}_GB_{b}", bufs=2)
                        if os.environ.get("GCN_DBG_NOGATHER", "0") == "1":
                            for _k in range(K_A):
                                nc.sync.dma_start(
                                    GA[:, _k * F_in:(_k + 1) * F_in],
                                    h_src[0:128, :])
                            for _k in range(K_B):
                                nc.sync.dma_start(
                                    GBt[:, _k * F_in:(_k + 1) * F_in],
                                    h_src[0:128, :])
                        else:
                            # ucode faults above 1024 idxs/call -> split
                            for c0 in range(0, K_A, 8):
                                c1 = min(K_A, c0 + 8)
                                n_i = (c1 - c0) * 128
                                nc.gpsimd.dma_gather(
                                    GA[:, c0 * F_in:c1 * F_in].rearrange(
                                        "p (c f) -> p c f", f=F_in),
                                    viewA,
                                    idxA_sb[:, b * K_A * 8 + c0 * 8:
                                            b * K_A * 8 + c1 * 8],
                                    n_i, n_i, F_in, elem_step=estep)
                            for c0 in range(0, K_B, 8):
                                c1 = min(K_B, c0 + 8)
                                n_i = (c1 - c0) * 128
                                nc.gpsimd.dma_gather(
                                    GBt[:, c0 * F_in:c1 * F_in].rearrange(
                                        "p (c f) -> p c f", f=F_in),
                                    viewB,
                                    idxB_sb[:, b * K_B * 8 + c0 * 8:
                                            b * K_B * 8 + c1 * 8],
                                    n_i, n_i, F_in, elem_step=estep)
                        P = ppool.tile([128, F_in], f32, tag="P",
                                       name=f"{lname}_P_{b}", bufs=2)
                        for k in range(KT):
                            Gsrc = GA if k < K_A else GBt
                            kk = k if k < K_A else k - K_A
                            for t in range(nch):
                                w_ = min(512, F_in - t * 512)
                                nc.tensor.matmul(
                                    out=P[:, t * 512:t * 512 + w_],
                                    lhsT=S_t[:, k * 128:(k + 1) * 128],
                                    rhs=Gsrc[:, kk * F_in + t * 512:
                                             kk * F_in + t * 512 + w_],
                                    start=(k == 0), stop=(k == KT - 1))
                        g_sb = wpool.tile([128, F_in], bf16, tag="g_sb",
                                          name=f"{lname}_gsb_{b}", bufs=2)
                        nc.vector.tensor_copy(out=g_sb[:], in_=P[:])
                        for k2 in range(FC_in):
                            T = ppool.tile([128, 128], bf16, tag="T",
                                           name=f"{lname}_T_{b}_{k2}", bufs=2)
                            nc.tensor.transpose(
                                out=T[:], in_=g_sb[:, k2 * 128:(k2 + 1) * 128],
                                identity=ident[:])
                            nc.vector.tensor_copy(
                                out=gT[:, k2 * GW + j * 128:
                                       k2 * GW + (j + 1) * 128],
                                in_=T[:])
                    # dense for this group
                    if not is_last:
                        for fo in range(FC_out):
                            O = ppool.tile([128, GW], f32, tag="O",
                                           name=f"{lname}_O_{g}_{fo}", bufs=2)
                            for k2 in range(FC_in):
                                nc.tensor.matmul(
                                    out=O[:],
                                    lhsT=W_sb[k2][:, fo * 128:(fo + 1) * 128],
                                    rhs=gT[:, k2 * GW:(k2 + 1) * GW],
                                    start=(k2 == 0), stop=(k2 == FC_in - 1))
                            oT = wpool.tile([128, GW], bf16, tag="oT",
                                            name=f"{lname}_oT_{g}_{fo}", bufs=2)
                            nc.scalar.activation(
                                oT[:], O[:], Copy,
                                accum_out=sum_acc[:, fo * ngrp + g:
                                                  fo * ngrp + g + 1])
                            sq = wpool.tile([128, GW], bf16, tag="sq",
                                            name=f"{lname}_sq_{g}_{fo}", bufs=2)
                            nc.scalar.activation(
                                sq[:], O[:], Square,
                                accum_out=sq_acc[:, fo * ngrp + g:
                                                 fo * ngrp + g + 1])
                            nc.sync.dma_start(
                                pre[fo * 128:(fo + 1) * 128,
                                    g * GW:(g + 1) * GW], oT[:])
                    else:
                        for j2 in range(GB):
                            orow = wpool.tile([128, OUT_PAD], bf16, tag="orow",
                                              name=f"o_{g}_{j2}", bufs=2)
                            for t6 in range(OUT_PAD // 512):
                                O = ppool.tile([128, 512], f32, tag="O",
                                               name=f"O3_{g}_{j2}_{t6}", bufs=2)
                                for k2 in range(FC_in):
                                    nc.tensor.matmul(
                                        out=O[:],
                                        lhsT=gT[:, k2 * GW + j2 * 128:
                                                k2 * GW + (j2 + 1) * 128],
                                        rhs=W3_sb[k2][:, t6 * 512:
                                                      (t6 + 1) * 512],
                                        start=(k2 == 0), stop=(k2 == FC_in - 1))
                                if t6 % 2 == 0:
                                    nc.vector.tensor_copy(
                                        out=orow[:, t6 * 512:(t6 + 1) * 512],
                                        in_=O[:])
                                else:
                                    nc.scalar.copy(
                                        out=orow[:, t6 * 512:(t6 + 1) * 512],
                                        in_=O[:])
                            bb = GB * g + j2
                            nc.sync.dma_start(
                                out_d[bb * 128:(bb + 1) * 128, :], orow[:])

                if is_last:
                    return

                # ------- BN stats: local reduce, AllReduce, scale/bias
                loc = cpool.tile([128, 2 * FC_out], f32, name=f"{lname}_loc")
                for fo in range(FC_out):
                    nc.vector.reduce_sum(
                        out=loc[:, fo:fo + 1],
                        in_=sum_acc[:, fo * ngrp:(fo + 1) * ngrp],
                        axis=mybir.AxisListType.X)
                    nc.vector.reduce_sum(
                        out=loc[:, FC_out + fo:FC_out + fo + 1],
                        in_=sq_acc[:, fo * ngrp:(fo + 1) * ngrp],
                        axis=mybir.AxisListType.X)
                nc.sync.dma_start(ar_in[:], loc[:])
                if os.environ.get("GCN_DBG_NOCOLL", "0") != "1":
                    nc.gpsimd.collective_compute(
                        "AllReduce", mybir.AluOpType.add, replica_groups=groups,
                        ins=[ar_in.opt()], outs=[ar_out.opt()])
                else:
                    nc.sync.dma_start(ar_out[:], ar_in[:])
                gst = cpool.tile([128, 2 * FC_out], f32, name=f"{lname}_gst")
                nc.sync.dma_start(gst[:], ar_out[:])
                m_t = cpool.tile([128, FC_out], f32, name=f"{lname}_m")
                nc.vector.tensor_scalar_mul(m_t[:], gst[:, :FC_out], INV_N)
                ex2 = cpool.tile([128, FC_out], f32, name=f"{lname}_ex2")
                nc.vector.tensor_scalar_mul(ex2[:], gst[:, FC_out:], INV_N)
                var = cpool.tile([128, FC_out], f32, name=f"{lname}_var")
                nc.vector.tensor_mul(var[:], m_t[:], m_t[:])
                nc.vector.tensor_sub(var[:], ex2[:], var[:])
                nc.vector.tensor_scalar_add(var[:], var[:], EPS)
                sd = cpool.tile([128, FC_out], f32, name=f"{lname}_sd")
                nc.scalar.activation(sd[:], var[:], Sqrt)
                rstd = cpool.tile([128, FC_out], f32, name=f"{lname}_rstd")
                nc.vector.reciprocal(rstd[:], sd[:])
                s_t = cpool.tile([128, FC_out], f32, name=f"{lname}_s")
                nc.vector.tensor_mul(s_t[:], rstd[:], gb_sb[:, :FC_out])
                t_t = cpool.tile([128, FC_out], f32, name=f"{lname}_t")
                nc.vector.tensor_mul(t_t[:], m_t[:], s_t[:])
                nc.vector.tensor_sub(t_t[:], gb_sb[:, FC_out:2 * FC_out], t_t[:])

                # ------- normalize + relu + transpose + writeback
                for g in range(ngrp):
                    hrs = [wpool.tile([128, F_out], bf16, tag="hr",
                                      name=f"{lname}_hr_{g}_{j}", bufs=GB)
                           for j in range(GB)]
                    for fo in range(FC_out):
                        pt = wpool.tile([128, GW], bf16, tag="pt",
                                        name=f"{lname}_pt_{g}_{fo}", bufs=2)
                        nc.sync.dma_start(
                            pt[:], pre[fo * 128:(fo + 1) * 128,
                                       g * GW:(g + 1) * GW])
                        hT = wpool.tile([128, GW], bf16, tag="hT",
                                        name=f"{lname}_hT_{g}_{fo}", bufs=2)
                        nc.scalar.activation(
                            hT[:], pt[:], Relu,
                            bias=t_t[:, fo:fo + 1], scale=s_t[:, fo:fo + 1])
                        for j in range(GB):
                            T2 = ppool.tile([128, 128], bf16, tag="T",
                                            name=f"{lname}_T2_{g}_{fo}_{j}",
                                            bufs=2)
                            nc.tensor.transpose(
                                out=T2[:], in_=hT[:, j * 128:(j + 1) * 128],
                                identity=ident[:])
                            nc.vector.tensor_copy(
                                out=hrs[j][:, fo * 128:(fo + 1) * 128],
                                in_=T2[:])
                    for j in range(GB):
                        bb = GB * g + j
                        nc.sync.dma_start(
                            ag_in[bb * 128:(bb + 1) * 128, :], hrs[j][:])
                if os.environ.get("GCN_DBG_NOCOLL", "0") != "1":
                    nc.gpsimd.collective_compute(
                        "AllGather", mybir.AluOpType.bypass,
                        replica_groups=groups,
                        ins=[ag_in.opt()], outs=[h_next.opt()])
                else:
                    for _r in range(ncores):
                        nc.sync.dma_start(
                            h_next[_r * nblk * 128:(_r + 1) * nblk * 128, :],
                            ag_in[:])

            dbg_layers = int(os.environ.get("GCN_DBG_LAYERS", "3"))
            layer(x_tab, F_IN, H1, W1_sb, pre1, ag1_in, h1,
                  ar1_in, ar1_out, gb1_sb, "L1")
            if dbg_layers >= 2:
                layer(h1, H1, H2, W2_sb, pre2, ag2_in, h2,
                      ar2_in, ar2_out, gb2_sb, "L2")
            if dbg_layers >= 3:
                layer(h2, H2, OUT_PAD, None, None, None, None,
                      None, None, None, "L3", is_last=True)

    nc.compile()
    return nc


# ---------------------------------------------------------------- runner

_CACHE = {}


def _make_in_maps(prep, inputs, cfg):
    W1 = np.asarray(inputs["W1"], np.float32).astype(BF16)
    W2 = np.asarray(inputs["W2"], np.float32).astype(BF16)
    W3p = np.zeros((cfg["H2"], cfg["OUT_PAD"]), np.float32)
    W3p[:, :cfg["OUT"]] = np.asarray(inputs["W3"], np.float32)
    W3p = W3p.astype(BF16)
    gb1 = np.concatenate(
        [_feat_t(inputs["g1"], cfg["H1"]), _feat_t(inputs["be1"], cfg["H1"])],
        axis=1)
    gb2 = np.concatenate(
        [_feat_t(inputs["g2"], cfg["H2"]), _feat_t(inputs["be2"], cfg["H2"])],
        axis=1)
    in_maps = []
    for c in range(cfg["NCORES"]):
        in_maps.append({
            "x_tab": prep["x_tab"],
            "idxA": prep["idxA_cores"][c],
            "idxB": prep["idxB_cores"][c],
            "S": prep["S_cores"][c],
            "W1": W1, "W2": W2, "W3": W3p,
            "gb1": gb1, "gb2": gb2,
        })
    return in_maps


def _postprocess(results, prep, inputs, cfg):
    out_cat = np.concatenate([r["out"] for r in results], axis=0)
    out = out_cat[prep["slot_of_node"], :cfg["OUT"]].astype(np.float32)
    out += np.asarray(inputs["b3"], np.float32)[None, :]
    return out


def prepare(inputs, cfg, verbose=False):
    """Preprocess + build (cached) + input staging; returns run context."""
    t0 = time.perf_counter()
    cfg = _derived(cfg)
    prep = _preprocess(inputs["x"], inputs["edge_index"], inputs["edge_attr"],
                       cfg)
    t1 = time.perf_counter()
    key = (cfg["N"], cfg["NP"], prep["K_A"], prep["K_B"])
    if key not in _CACHE:
        _CACHE[key] = _build(cfg, prep["K_A"], prep["K_B"])
    nc = _CACHE[key]
    t2 = time.perf_counter()
    in_maps = _make_in_maps(prep, inputs, cfg)
    t3 = time.perf_counter()
    if verbose:
        print(f"[kernel] prep={t1-t0:.1f}s KA={prep['K_A']} KB={prep['K_B']} "
              f"build={t2-t1:.1f}s inmaps={t3-t2:.1f}s", flush=True)
    return dict(cfg=cfg, prep=prep, nc=nc, in_maps=in_maps)


def run_cfg(inputs, cfg, verbose=False):
    """Full pipeline for an arbitrary cfg (used by tests and kernel())."""
    ctx = prepare(inputs, cfg, verbose=verbose)
    t3 = time.perf_counter()
    from concourse.bass_utils import run_bass_kernel_spmd
    r = run_bass_kernel_spmd(ctx["nc"], ctx["in_maps"],
                             core_ids=list(range(ctx["cfg"]["NCORES"])))
    t4 = time.perf_counter()
    out = _postprocess(r.results, ctx["prep"], inputs, ctx["cfg"])
    t5 = time.perf_counter()
    if verbose:
        print(f"[kernel] run={t4-t3:.1f}s post={t5-t4:.1f}s", flush=True)
    return out


def kernel(x, edge_index, edge_attr, y, W1, b1, g1, be1, W2, b2, g2, be2,
           W3, b3):
    inputs = dict(x=x, edge_index=edge_index, edge_attr=edge_attr, y=y,
                  W1=W1, b1=b1, g1=g1, be1=be1, W2=W2, b2=b2, g2=g2, be2=be2,
                  W3=W3, b3=b3)
    return run_cfg(inputs, REAL_CFG)


# ---------------------------------------------------------------- timing

def make_runner(nc, in_maps, n_cores):
    """Compile once, return a callable that executes the NEFF on device-
    resident inputs and reports per-call wall times (dispatch + execute)."""
    import jax
    from jax.experimental.shard_map import shard_map
    from jax.sharding import Mesh, NamedSharding, PartitionSpec
    import concourse.mybir as mybir
    from concourse import bass2jax

    bass2jax.install_neuronx_cc_hook()
    assert nc.dbg_addr is None
    partition_name = (nc.partition_id_tensor.name
                      if nc.partition_id_tensor else None)

    in_names, out_names, out_avals, zero_outs = [], [], [], []
    for alloc in nc.m.functions[0].allocations:
        if not isinstance(alloc, mybir.MemoryLocationSet):
            continue
        name = alloc.memorylocations[0].name
        if alloc.kind == "ExternalInput":
            if name != partition_name:
                in_names.append(name)
        elif alloc.kind == "ExternalOutput":
            shape = tuple(alloc.tensor_shape)
            dtype = mybir.dt.np(alloc.dtype)
            out_names.append(name)
            out_avals.append(jax.core.ShapedArray(shape, dtype))
            zero_outs.append((shape, dtype))
    n_params = len(in_names)
    n_outs = len(out_avals)
    all_names = in_names + out_names
    if partition_name is not None:
        all_names = all_names + [partition_name]
    donate = tuple(range(n_params, n_params + n_outs))

    def _body(*args):
        operands = list(args)
        if partition_name is not None:
            operands.append(bass2jax.partition_id_tensor())
        outs = bass2jax._bass_exec_p.bind(
            *operands,
            out_avals=tuple(out_avals),
            in_names=tuple(all_names),
            out_names=tuple(out_names),
            lowering_input_output_aliases=(),
            sim_require_finite=True,
            sim_require_nnan=True,
            nc=nc,
        )
        return tuple(outs)

    devices = jax.devices()[:n_cores]
    mesh = Mesh(np.asarray(devices), ("core",))
    in_specs = (PartitionSpec("core"),) * (n_params + n_outs)
    out_specs = (PartitionSpec("core"),) * n_outs
    sharded = jax.jit(
        shard_map(_body, mesh=mesh, in_specs=in_specs, out_specs=out_specs,
                  check_rep=False),
        donate_argnums=donate, keep_unused=True,
    )
    sh = NamedSharding(mesh, PartitionSpec("core"))
    concat_in = [
        jax.device_put(
            np.concatenate([np.asarray(in_maps[c][k])
                            for c in range(n_cores)], axis=0), sh)
        for k in in_names
    ]

    def run(n_repeats=1):
        times = []
        outs = None
        for _ in range(n_repeats):
            z = [jax.device_put(np.zeros((n_cores * s0[0], *s0[1:]), d0), sh)
                 for s0, d0 in zero_outs]
            jax.block_until_ready(z)
            t0 = time.perf_counter()
            outs = sharded(*concat_in, *z)
            jax.block_until_ready(outs)
            times.append(time.perf_counter() - t0)
        results = [
            {k: np.asarray(outs[i]).reshape(n_cores, *out_avals[i].shape)[c]
             for i, k in enumerate(out_names)}
            for c in range(n_cores)
        ]
        return results, times

    return run


# revision 14
# speedup vs baseline: 1.2666x; 1.2666x over previous
"""GCNDecoder on 8 Trainium2 NeuronCores (Bass/Tile).

3-layer GCN: (GCNConv -> BN -> ReLU) x2 -> GCNConv, N=50000 nodes, E=800000
edges, feature dims 256 -> 512 -> 1024 -> 3000.

Strategy (data-parallel over nodes, per the sharding hint):
  * Reassociate each layer as (A_hat @ h) @ W: aggregate FIRST in the smaller
    input-feature dim, then dense-matmul.  A_hat includes self loops.
  * Nodes are permuted and padded to 53248 = 8 cores x 52 blocks x 128 so
    every core owns an equal shard; a greedy 2-D bin-pack equalizes per-block
    in-edge counts, so the SPMD program is identical on all cores.
  * Edge aggregation: gather h[src] rows with the GPSIMD dma_gather ucode
    (int16 indices -> the node table is split by slot parity into two
    strided views of <=26624 rows each), then segment-sum via TensorEngine
    matmuls against host-precomputed one-hot*coef matrices (S).
  * Dense matmul produces the TRANSPOSED activation [F_out, nodes] so that
    BatchNorm stats (free-dim reduce + per-partition affine) are native; the
    scalar engine's accum_out gives row sums for free.
  * BN statistics: AllReduce of per-core (sum, sumsq); the BN beta/bias of
    the reference cancels in BN so layer biases b1, b2 are dropped; b3 is
    added on the host.  BN renormalizes, relu, transpose back to row-major,
    then AllGather re-replicates the node table for the next layer's gather.
  * Everything on-device is bf16 with fp32 PSUM accumulation (rel-err budget
    2e-2).
"""

import os
import time

import numpy as np
import ml_dtypes

BF16 = ml_dtypes.bfloat16

# ---------------------------------------------------------------- config

REAL_CFG = dict(
    N=50000, F_IN=256, H1=512, H2=1024, OUT=3000, OUT_PAD=3072,
    NCORES=8, BLOCKS_PER_CORE=52, GROUP_BLOCKS=4, EPS=1e-5,
)


def _derived(cfg):
    cfg = dict(cfg)
    cfg["NP"] = cfg["NCORES"] * cfg["BLOCKS_PER_CORE"] * 128
    cfg["NGRP"] = cfg["BLOCKS_PER_CORE"] // cfg["GROUP_BLOCKS"]
    assert cfg["BLOCKS_PER_CORE"] % cfg["GROUP_BLOCKS"] == 0
    return cfg


# ---------------------------------------------------------------- host prep

def _preprocess(x, edge_index, edge_attr, cfg):
    """Permute/pad nodes, classify+pack edges, build S / index tables."""
    N, NP = cfg["N"], cfg["NP"]
    nb_tot = NP // 128

    src = np.asarray(edge_index[0], dtype=np.int64)
    dst = np.asarray(edge_index[1], dtype=np.int64)
    w = np.asarray(edge_attr, dtype=np.float32)[:, 2]

    deg = np.bincount(dst, weights=w, minlength=N).astype(np.float32) + 1.0
    dinv = 1.0 / np.sqrt(deg)
    coef = (dinv[src] * w * dinv[dst]).astype(np.float32)
    selfc = (dinv * dinv).astype(np.float32)

    # ---- parity assignment: balance total out-degree between parities
    out_deg = np.bincount(src, minlength=N)
    order = np.argsort(-out_deg, kind="stable")
    parity = np.empty(N, np.int8)
    parity[order] = (np.arange(N) % 2).astype(np.int8)

    # per-node in-degree counts by src parity (slot counts, unweighted)
    a_deg = np.bincount(dst[parity[src] == 0], minlength=N)
    b_deg = np.bincount(dst[parity[src] == 1], minlength=N)

    # ---- greedy 2-D bin pack: nodes -> blocks (64 even + 64 odd slots each)
    loadA = np.zeros(nb_tot, np.float64)
    loadB = np.zeros(nb_tot, np.float64)
    quota = np.zeros((nb_tot, 2), np.int32)  # filled count per parity
    blk_of_node = np.empty(N, np.int32)
    nodes_by_sz = np.argsort(-(a_deg + b_deg), kind="stable")
    INF = 1e18
    for v in nodes_by_sz:
        p = parity[v]
        da = a_deg[v] + (1 - p)  # own self-edge contributes to its parity class
        db = b_deg[v] + p
        cost = np.maximum(loadA + da, loadB + db)
        cost[quota[:, p] >= 64] = INF
        b = int(np.argmin(cost))
        blk_of_node[v] = b
        loadA[b] += da
        loadB[b] += db
        quota[b, p] += 1

    # ---- slots: even-parity nodes at even slots, odd at odd
    slot_of_node = np.empty(N, np.int64)
    fill = np.zeros((nb_tot, 2), np.int64)
    # stable order within block: iterate nodes grouped by (block, parity)
    order2 = np.lexsort((parity, blk_of_node))
    for v in order2:
        b = blk_of_node[v]
        p = parity[v]
        slot_of_node[v] = b * 128 + 2 * fill[b, p] + p
        fill[b, p] += 1

    # ---- edge arrays (real edges + self edges)
    e_src_slot = slot_of_node[src]
    e_dst_slot = slot_of_node[dst]
    all_src = np.concatenate([e_src_slot, slot_of_node])
    all_dst = np.concatenate([e_dst_slot, slot_of_node])
    all_coef = np.concatenate([coef, selfc])

    e_cls = (all_src % 2).astype(np.int8)  # 0 = table A (even), 1 = B
    e_tab = (all_src // 2).astype(np.int64)
    e_blk = (all_dst // 128).astype(np.int64)
    e_dloc = (all_dst % 128).astype(np.int64)

    # rank within (block, class)
    sidx = np.lexsort((e_cls, e_blk))
    sb, sc = e_blk[sidx], e_cls[sidx]
    key = sb * 2 + sc
    _, starts, counts = np.unique(key, return_index=True, return_counts=True)
    rank = np.arange(len(sidx)) - np.repeat(starts, counts)
    nA = np.zeros(nb_tot, np.int64)
    nB = np.zeros(nb_tot, np.int64)
    kk = np.zeros(nb_tot * 2, np.int64)
    kk[np.unique(key)] = counts
    nA = kk[0::2]
    nB = kk[1::2]
    K_A = int(np.ceil(nA.max() / 128))
    K_B = int(np.ceil(nB.max() / 128))
    KT = K_A + K_B

    # ---- per-core packed structures
    nblk = cfg["BLOCKS_PER_CORE"]
    ncores = cfg["NCORES"]
    S_cores, idxA_cores, idxB_cores = [], [], []
    bl_l = (sb % nblk).astype(np.int64)
    core_l = (sb // nblk).astype(np.int64)
    chunk = np.where(sc == 0, rank // 128, K_A + rank // 128)
    part = rank % 128
    s_coef = all_coef[sidx].astype(np.float32)
    s_dloc = e_dloc[sidx]
    s_tab = e_tab[sidx]
    for c in range(ncores):
        m = core_l == c
        S = np.zeros((nblk, 128, KT * 128), np.float32)
        S[bl_l[m], part[m], chunk[m] * 128 + s_dloc[m]] = s_coef[m]
        S_cores.append(S.astype(BF16))

        idxA = np.zeros((nblk, K_A * 128), np.int16)
        idxB = np.zeros((nblk, K_B * 128), np.int16)
        ma = m & (sc == 0)
        mb = m & (sc == 1)
        idxA[bl_l[ma], rank[ma]] = s_tab[ma].astype(np.int16)
        idxB[bl_l[mb], rank[mb]] = s_tab[mb].astype(np.int16)

        def pack(idx, K):
            # slot i of each block -> [i % 16, i // 16], replicated x8
            t = idx.reshape(nblk, K * 8, 16).transpose(2, 0, 1).reshape(16, -1)
            return np.tile(t, (8, 1)).copy()

        idxA_cores.append(pack(idxA, K_A))
        idxB_cores.append(pack(idxB, K_B))

    # ---- node table for layer 1
    x = np.asarray(x, dtype=np.float32)
    x_tab = np.zeros((NP, cfg["F_IN"]), BF16)
    x_tab[slot_of_node] = x.astype(BF16)

    return dict(
        K_A=K_A, K_B=K_B, KT=KT, slot_of_node=slot_of_node,
        S_cores=S_cores, idxA_cores=idxA_cores, idxB_cores=idxB_cores,
        x_tab=x_tab,
    )


def _feat_t(v, F):
    """[F] per-feature vector -> [128, F//128] tile layout (f at [f%128, f//128])."""
    v = np.asarray(v, dtype=np.float32)
    out = np.zeros((128, F // 128), np.float32)
    out[:, :] = v.reshape(F // 128, 128).T
    return out


# ---------------------------------------------------------------- builder

def _build(cfg, K_A, K_B, repeats=1):
    import concourse.bass as bass
    import concourse.bacc as bacc
    import concourse.mybir as mybir
    import concourse.tile as tile
    from concourse.masks import make_identity

    NP = cfg["NP"]
    F_IN, H1, H2, OUT_PAD = cfg["F_IN"], cfg["H1"], cfg["H2"], cfg["OUT_PAD"]
    nblk, GB, ngrp = cfg["BLOCKS_PER_CORE"], cfg["GROUP_BLOCKS"], cfg["NGRP"]
    ncores = cfg["NCORES"]
    KT = K_A + K_B
    GW = GB * 128  # node-group width
    INV_N = 1.0 / cfg["N"]
    EPS = cfg["EPS"]
    groups = [list(range(ncores))]
    f32 = mybir.dt.float32
    bf16 = mybir.dt.bfloat16

    nc = bacc.Bacc("TRN2", target_bir_lowering=False, debug=False,
                   num_devices=ncores)

    x_tab = nc.dram_tensor("x_tab", [NP, F_IN], bf16, kind="ExternalInput")
    idxA_d = nc.dram_tensor("idxA", [128, nblk * K_A * 8], mybir.dt.int16,
                            kind="ExternalInput")
    idxB_d = nc.dram_tensor("idxB", [128, nblk * K_B * 8], mybir.dt.int16,
                            kind="ExternalInput")
    S_d = nc.dram_tensor("S", [nblk, 128, KT * 128], bf16, kind="ExternalInput")
    W1_d = nc.dram_tensor("W1", [F_IN, H1], bf16, kind="ExternalInput")
    W2_d = nc.dram_tensor("W2", [H1, H2], bf16, kind="ExternalInput")
    W3_d = nc.dram_tensor("W3", [H2, OUT_PAD], bf16, kind="ExternalInput")
    gb1_d = nc.dram_tensor("gb1", [128, 2 * (H1 // 128)], f32,
                           kind="ExternalInput")
    gb2_d = nc.dram_tensor("gb2", [128, 2 * (H2 // 128)], f32,
                           kind="ExternalInput")
    out_d = nc.dram_tensor("out", [nblk * 128, OUT_PAD], bf16,
                           kind="ExternalOutput")

    with tile.TileContext(nc) as tc:
        with (
            tc.tile_pool(name="const", bufs=1) as cpool,
            tc.tile_pool(name="work", bufs=2) as wpool,
            tc.tile_pool(name="psum", bufs=2, space="PSUM") as ppool,
            tc.tile_pool(name="dram", bufs=1, space="DRAM") as dpool,
        ):
            # ------- resident tiles
            ident = cpool.tile([128, 128], bf16, name="ident")
            make_identity(nc, ident[:])
            idxA_sb = cpool.tile([128, nblk * K_A * 8], mybir.dt.int16,
                                 name="idxA_sb")
            nc.sync.dma_start(idxA_sb[:], idxA_d[:])
            idxB_sb = cpool.tile([128, nblk * K_B * 8], mybir.dt.int16,
                                 name="idxB_sb")
            nc.sync.dma_start(idxB_sb[:], idxB_d[:])

            def load_w(wd, fin, fout, name):
                tiles = []
                for k2 in range(fin // 128):
                    t = cpool.tile([128, fout], bf16, name=f"{name}_{k2}")
                    nc.sync.dma_start(t[:], wd[k2 * 128:(k2 + 1) * 128, :])
                    tiles.append(t)
                return tiles

            W1_sb = load_w(W1_d, F_IN, H1, "w1")
            W2_sb = load_w(W2_d, H1, H2, "w2")
            W3_sb = load_w(W3_d, H2, OUT_PAD, "w3")
            gb1_sb = cpool.tile([128, 2 * (H1 // 128)], f32, name="gb1_sb")
            nc.sync.dma_start(gb1_sb[:], gb1_d[:])
            gb2_sb = cpool.tile([128, 2 * (H2 // 128)], f32, name="gb2_sb")
            nc.sync.dma_start(gb2_sb[:], gb2_d[:])

            # ------- internal DRAM
            pre1 = dpool.tile([H1, nblk * 128], bf16, name="pre1")
            pre2 = dpool.tile([H2, nblk * 128], bf16, name="pre2")
            ag1_in = dpool.tile([nblk * 128, H1], bf16, name="ag1_in")
            ag2_in = dpool.tile([nblk * 128, H2], bf16, name="ag2_in")
            _shared = ("Local" if os.environ.get("GCN_DBG_NOCOLL", "0") == "1"
                       else "Shared")
            h1 = dpool.tile([NP, H1], bf16, name="h1", addr_space=_shared)
            h2 = dpool.tile([NP, H2], bf16, name="h2", addr_space=_shared)
            ar1_in = dpool.tile([128, 2 * (H1 // 128)], f32, name="ar1_in")
            ar1_out = dpool.tile([128, 2 * (H1 // 128)], f32, name="ar1_out",
                                 addr_space="Shared")
            ar2_in = dpool.tile([128, 2 * (H2 // 128)], f32, name="ar2_in")
            ar2_out = dpool.tile([128, 2 * (H2 // 128)], f32, name="ar2_out",
                                 addr_space="Shared")

            Copy = mybir.ActivationFunctionType.Copy
            Square = mybir.ActivationFunctionType.Square
            Relu = mybir.ActivationFunctionType.Relu
            Sqrt = mybir.ActivationFunctionType.Sqrt

            def layer(h_src, F_in, F_out, W_sb, pre, ag_in, h_next,
                      ar_in, ar_out, gb_sb, lname, is_last=False):
                FC_in = F_in // 128
                FC_out = F_out // 128
                nch = (F_in + 511) // 512  # agg matmul N-chunks

                if os.environ.get("GCN_DBG_NOSTRIDE", "0") == "1":
                    viewA = h_src[0:NP // 2, :]
                    viewB = h_src[NP // 2:, :]
                    estep = F_in
                else:
                    hv = h_src[:].rearrange("(n two) f -> two n f", two=2)
                    viewA, viewB = hv[0], hv[1]
                    estep = 2 * F_in

                if not is_last:
                    sum_acc = cpool.tile([128, FC_out * ngrp], f32,
                                         name=f"{lname}_sum")
                    sq_acc = cpool.tile([128, FC_out * ngrp], f32,
                                        name=f"{lname}_sq")

                for g in range(ngrp):
                    gT = wpool.tile([128, FC_in * GW], bf16, tag="gT",
                                    name=f"{lname}_gT_{g}", bufs=2)
                    for j in range(GB):
                        b = GB * g + j
                        S_t = wpool.tile([128, KT * 128], bf16, tag="S_t",
                                         name=f"{lname}_S_{b}", bufs=2)
                        nc.sync.dma_start(S_t[:], S_d[b])
                        GA = wpool.tile([128, K_A * F_in], bf16, tag="GA",
                                        name=f"{lname}_GA_{b}", bufs=2)
                        GBt = wpool.tile([128, K_B * F_in], bf16, tag="GB",
                                         name=f"{lname}_GB_{b}", bufs=2)
                        if os.environ.get("GCN_DBG_NOGATHER", "0") == "1":
                            for _k in range(K_A):
                                nc.sync.dma_start(
                                    GA[:, _k * F_in:(_k + 1) * F_in],
                                    h_src[0:128, :])
                            for _k in range(K_B):
                                nc.sync.dma_start(
                                    GBt[:, _k * F_in:(_k + 1) * F_in],
                                    h_src[0:128, :])
                        else:
                            # ucode faults above 1024 idxs/call -> split
                            for c0 in range(0, K_A, 8):
                                c1 = min(K_A, c0 + 8)
                                n_i = (c1 - c0) * 128
                                nc.gpsimd.dma_gather(
                                    GA[:, c0 * F_in:c1 * F_in].rearrange(
                                        "p (c f) -> p c f", f=F_in),
                                    viewA,
                                    idxA_sb[:, b * K_A * 8 + c0 * 8:
                                            b * K_A * 8 + c1 * 8],
                                    n_i, n_i, F_in, elem_step=estep)
                            for c0 in range(0, K_B, 8):
                                c1 = min(K_B, c0 + 8)
                                n_i = (c1 - c0) * 128
                                nc.gpsimd.dma_gather(
                                    GBt[:, c0 * F_in:c1 * F_in].rearrange(
                                        "p (c f) -> p c f", f=F_in),
                                    viewB,
                                    idxB_sb[:, b * K_B * 8 + c0 * 8:
                                            b * K_B * 8 + c1 * 8],
                                    n_i, n_i, F_in, elem_step=estep)
                        P = ppool.tile([128, F_in], f32, tag="P",
                                       name=f"{lname}_P_{b}", bufs=2)
                        for k in range(KT):
                            Gsrc = GA if k < K_A else GBt
                            kk = k if k < K_A else k - K_A
                            for t in range(nch):
                                w_ = min(512, F_in - t * 512)
                                nc.tensor.matmul(
                                    out=P[:, t * 512:t * 512 + w_],
                                    lhsT=S_t[:, k * 128:(k + 1) * 128],
                                    rhs=Gsrc[:, kk * F_in + t * 512:
                                             kk * F_in + t * 512 + w_],
                                    start=(k == 0), stop=(k == KT - 1))
                        g_sb = wpool.tile([128, F_in], bf16, tag="g_sb",
                                          name=f"{lname}_gsb_{b}", bufs=2)
                        nc.vector.tensor_copy(out=g_sb[:], in_=P[:])
                        for k2 in range(FC_in):
                            T = ppool.tile([128, 128], bf16, tag="T",
                                           name=f"{lname}_T_{b}_{k2}", bufs=2)
                            nc.tensor.transpose(
                                out=T[:], in_=g_sb[:, k2 * 128:(k2 + 1) * 128],
                                identity=ident[:])
                            nc.vector.tensor_copy(
                                out=gT[:, k2 * GW + j * 128:
                                       k2 * GW + (j + 1) * 128],
                                in_=T[:])
                    # dense for this group
                    if not is_last:
                        for fo in range(FC_out):
                            O = ppool.tile([128, GW], f32, tag="O",
                                           name=f"{lname}_O_{g}_{fo}", bufs=2)
                            for k2 in range(FC_in):
                                nc.tensor.matmul(
                                    out=O[:],
                                    lhsT=W_sb[k2][:, fo * 128:(fo + 1) * 128],
                                    rhs=gT[:, k2 * GW:(k2 + 1) * GW],
                                    start=(k2 == 0), stop=(k2 == FC_in - 1))
                            oT = wpool.tile([128, GW], bf16, tag="oT",
                                            name=f"{lname}_oT_{g}_{fo}", bufs=2)
                            nc.scalar.activation(
                                oT[:], O[:], Copy,
                                accum_out=sum_acc[:, fo * ngrp + g:
                                                  fo * ngrp + g + 1])
                            sq = wpool.tile([128, GW], bf16, tag="sq",
                                            name=f"{lname}_sq_{g}_{fo}", bufs=2)
                            nc.scalar.activation(
                                sq[:], O[:], Square,
                                accum_out=sq_acc[:, fo * ngrp + g:
                                                 fo * ngrp + g + 1])
                            nc.sync.dma_start(
                                pre[fo * 128:(fo + 1) * 128,
                                    g * GW:(g + 1) * GW], oT[:])
                    else:
                        for j2 in range(GB):
                            orow = wpool.tile([128, OUT_PAD], bf16, tag="orow",
                                              name=f"o_{g}_{j2}", bufs=2)
                            for t6 in range(OUT_PAD // 512):
                                O = ppool.tile([128, 512], f32, tag="O",
                                               name=f"O3_{g}_{j2}_{t6}", bufs=2)
                                for k2 in range(FC_in):
                                    nc.tensor.matmul(
                                        out=O[:],
                                        lhsT=gT[:, k2 * GW + j2 * 128:
                                                k2 * GW + (j2 + 1) * 128],
                                        rhs=W3_sb[k2][:, t6 * 512:
                                                      (t6 + 1) * 512],
                                        start=(k2 == 0), stop=(k2 == FC_in - 1))
                                if t6 % 2 == 0:
                                    nc.vector.tensor_copy(
                                        out=orow[:, t6 * 512:(t6 + 1) * 512],
                                        in_=O[:])
                                else:
                                    nc.scalar.copy(
                                        out=orow[:, t6 * 512:(t6 + 1) * 512],
                                        in_=O[:])
                            bb = GB * g + j2
                            nc.sync.dma_start(
                                out_d[bb * 128:(bb + 1) * 128, :], orow[:])

                if is_last:
                    return

                # ------- BN stats: local reduce, AllReduce, scale/bias
                loc = cpool.tile([128, 2 * FC_out], f32, name=f"{lname}_loc")
                for fo in range(FC_out):
                    nc.vector.reduce_sum(
                        out=loc[:, fo:fo + 1],
                        in_=sum_acc[:, fo * ngrp:(fo + 1) * ngrp],
                        axis=mybir.AxisListType.X)
                    nc.vector.reduce_sum(
                        out=loc[:, FC_out + fo:FC_out + fo + 1],
                        in_=sq_acc[:, fo * ngrp:(fo + 1) * ngrp],
                        axis=mybir.AxisListType.X)
                nc.sync.dma_start(ar_in[:], loc[:])
                if os.environ.get("GCN_DBG_NOCOLL", "0") != "1":
                    nc.gpsimd.collective_compute(
                        "AllReduce", mybir.AluOpType.add, replica_groups=groups,
                        ins=[ar_in.opt()], outs=[ar_out.opt()])
                else:
                    nc.sync.dma_start(ar_out[:], ar_in[:])
                gst = cpool.tile([128, 2 * FC_out], f32, name=f"{lname}_gst")
                nc.sync.dma_start(gst[:], ar_out[:])
                m_t = cpool.tile([128, FC_out], f32, name=f"{lname}_m")
                nc.vector.tensor_scalar_mul(m_t[:], gst[:, :FC_out], INV_N)
                ex2 = cpool.tile([128, FC_out], f32, name=f"{lname}_ex2")
                nc.vector.tensor_scalar_mul(ex2[:], gst[:, FC_out:], INV_N)
                var = cpool.tile([128, FC_out], f32, name=f"{lname}_var")
                nc.vector.tensor_mul(var[:], m_t[:], m_t[:])
                nc.vector.tensor_sub(var[:], ex2[:], var[:])
                nc.vector.tensor_scalar_add(var[:], var[:], EPS)
                sd = cpool.tile([128, FC_out], f32, name=f"{lname}_sd")
                nc.scalar.activation(sd[:], var[:], Sqrt)
                rstd = cpool.tile([128, FC_out], f32, name=f"{lname}_rstd")
                nc.vector.reciprocal(rstd[:], sd[:])
                s_t = cpool.tile([128, FC_out], f32, name=f"{lname}_s")
                nc.vector.tensor_mul(s_t[:], rstd[:], gb_sb[:, :FC_out])
                t_t = cpool.tile([128, FC_out], f32, name=f"{lname}_t")
                nc.vector.tensor_mul(t_t[:], m_t[:], s_t[:])
                nc.vector.tensor_sub(t_t[:], gb_sb[:, FC_out:2 * FC_out], t_t[:])

                # ------- normalize + relu + transpose + writeback
                for g in range(ngrp):
                    hrs = [wpool.tile([128, F_out], bf16, tag="hr",
                                      name=f"{lname}_hr_{g}_{j}", bufs=GB)
                           for j in range(GB)]
                    for fo in range(FC_out):
                        pt = wpool.tile([128, GW], bf16, tag="pt",
                                        name=f"{lname}_pt_{g}_{fo}", bufs=2)
                        nc.sync.dma_start(
                            pt[:], pre[fo * 128:(fo + 1) * 128,
                                       g * GW:(g + 1) * GW])
                        hT = wpool.tile([128, GW], bf16, tag="hT",
                                        name=f"{lname}_hT_{g}_{fo}", bufs=2)
                        nc.scalar.activation(
                            hT[:], pt[:], Relu,
                            bias=t_t[:, fo:fo + 1], scale=s_t[:, fo:fo + 1])
                        for j in range(GB):
                            T2 = ppool.tile([128, 128], bf16, tag="T",
                                            name=f"{lname}_T2_{g}_{fo}_{j}",
                                            bufs=2)
                            nc.tensor.transpose(
                                out=T2[:], in_=hT[:, j * 128:(j + 1) * 128],
                                identity=ident[:])
                            nc.vector.tensor_copy(
                                out=hrs[j][:, fo * 128:(fo + 1) * 128],
                                in_=T2[:])
                    for j in range(GB):
                        bb = GB * g + j
                        nc.sync.dma_start(
                            ag_in[bb * 128:(bb + 1) * 128, :], hrs[j][:])
                if os.environ.get("GCN_DBG_NOCOLL", "0") != "1":
                    nc.gpsimd.collective_compute(
                        "AllGather", mybir.AluOpType.bypass,
                        replica_groups=groups,
                        ins=[ag_in.opt()], outs=[h_next.opt()])
                else:
                    for _r in range(ncores):
                        nc.sync.dma_start(
                            h_next[_r * nblk * 128:(_r + 1) * nblk * 128, :],
                            ag_in[:])

            dbg_layers = int(os.environ.get("GCN_DBG_LAYERS", "3"))
            for rep in range(repeats):
                sfx = f"r{rep}" if repeats > 1 else ""
                layer(x_tab, F_IN, H1, W1_sb, pre1, ag1_in, h1,
                      ar1_in, ar1_out, gb1_sb, "L1" + sfx)
                if dbg_layers >= 2:
                    layer(h1, H1, H2, W2_sb, pre2, ag2_in, h2,
                          ar2_in, ar2_out, gb2_sb, "L2" + sfx)
                if dbg_layers >= 3:
                    layer(h2, H2, OUT_PAD, None, None, None, None,
                          None, None, None, "L3" + sfx, is_last=True)

    nc.compile()
    return nc


# ---------------------------------------------------------------- runner

_CACHE = {}


def _make_in_maps(prep, inputs, cfg):
    W1 = np.asarray(inputs["W1"], np.float32).astype(BF16)
    W2 = np.asarray(inputs["W2"], np.float32).astype(BF16)
    W3p = np.zeros((cfg["H2"], cfg["OUT_PAD"]), np.float32)
    W3p[:, :cfg["OUT"]] = np.asarray(inputs["W3"], np.float32)
    W3p = W3p.astype(BF16)
    gb1 = np.concatenate(
        [_feat_t(inputs["g1"], cfg["H1"]), _feat_t(inputs["be1"], cfg["H1"])],
        axis=1)
    gb2 = np.concatenate(
        [_feat_t(inputs["g2"], cfg["H2"]), _feat_t(inputs["be2"], cfg["H2"])],
        axis=1)
    in_maps = []
    for c in range(cfg["NCORES"]):
        in_maps.append({
            "x_tab": prep["x_tab"],
            "idxA": prep["idxA_cores"][c],
            "idxB": prep["idxB_cores"][c],
            "S": prep["S_cores"][c],
            "W1": W1, "W2": W2, "W3": W3p,
            "gb1": gb1, "gb2": gb2,
        })
    return in_maps


def _postprocess(results, prep, inputs, cfg):
    out_cat = np.concatenate([r["out"] for r in results], axis=0)
    out = out_cat[prep["slot_of_node"], :cfg["OUT"]].astype(np.float32)
    out += np.asarray(inputs["b3"], np.float32)[None, :]
    return out


def prepare(inputs, cfg, verbose=False, repeats=1):
    """Preprocess + build (cached) + input staging; returns run context."""
    t0 = time.perf_counter()
    cfg = _derived(cfg)
    prep = _preprocess(inputs["x"], inputs["edge_index"], inputs["edge_attr"],
                       cfg)
    t1 = time.perf_counter()
    key = (cfg["N"], cfg["NP"], prep["K_A"], prep["K_B"], repeats)
    if key not in _CACHE:
        _CACHE[key] = _build(cfg, prep["K_A"], prep["K_B"], repeats=repeats)
    nc = _CACHE[key]
    t2 = time.perf_counter()
    in_maps = _make_in_maps(prep, inputs, cfg)
    t3 = time.perf_counter()
    if verbose:
        print(f"[kernel] prep={t1-t0:.1f}s KA={prep['K_A']} KB={prep['K_B']} "
              f"build={t2-t1:.1f}s inmaps={t3-t2:.1f}s", flush=True)
    return dict(cfg=cfg, prep=prep, nc=nc, in_maps=in_maps)


def run_cfg(inputs, cfg, verbose=False):
    """Full pipeline for an arbitrary cfg (used by tests and kernel())."""
    ctx = prepare(inputs, cfg, verbose=verbose)
    t3 = time.perf_counter()
    from concourse.bass_utils import run_bass_kernel_spmd
    r = run_bass_kernel_spmd(ctx["nc"], ctx["in_maps"],
                             core_ids=list(range(ctx["cfg"]["NCORES"])))
    t4 = time.perf_counter()
    out = _postprocess(r.results, ctx["prep"], inputs, ctx["cfg"])
    t5 = time.perf_counter()
    if verbose:
        print(f"[kernel] run={t4-t3:.1f}s post={t5-t4:.1f}s", flush=True)
    return out


def kernel(x, edge_index, edge_attr, y, W1, b1, g1, be1, W2, b2, g2, be2,
           W3, b3):
    inputs = dict(x=x, edge_index=edge_index, edge_attr=edge_attr, y=y,
                  W1=W1, b1=b1, g1=g1, be1=be1, W2=W2, b2=b2, g2=g2, be2=be2,
                  W3=W3, b3=b3)
    return run_cfg(inputs, REAL_CFG)


# ---------------------------------------------------------------- timing

def make_runner(nc, in_maps, n_cores):
    """Compile once, return a callable that executes the NEFF on device-
    resident inputs and reports per-call wall times (dispatch + execute)."""
    import jax
    from jax.experimental.shard_map import shard_map
    from jax.sharding import Mesh, NamedSharding, PartitionSpec
    import concourse.mybir as mybir
    from concourse import bass2jax

    bass2jax.install_neuronx_cc_hook()
    assert nc.dbg_addr is None
    partition_name = (nc.partition_id_tensor.name
                      if nc.partition_id_tensor else None)

    in_names, out_names, out_avals, zero_outs = [], [], [], []
    for alloc in nc.m.functions[0].allocations:
        if not isinstance(alloc, mybir.MemoryLocationSet):
            continue
        name = alloc.memorylocations[0].name
        if alloc.kind == "ExternalInput":
            if name != partition_name:
                in_names.append(name)
        elif alloc.kind == "ExternalOutput":
            shape = tuple(alloc.tensor_shape)
            dtype = mybir.dt.np(alloc.dtype)
            out_names.append(name)
            out_avals.append(jax.core.ShapedArray(shape, dtype))
            zero_outs.append((shape, dtype))
    n_params = len(in_names)
    n_outs = len(out_avals)
    all_names = in_names + out_names
    if partition_name is not None:
        all_names = all_names + [partition_name]
    donate = tuple(range(n_params, n_params + n_outs))

    def _body(*args):
        operands = list(args)
        if partition_name is not None:
            operands.append(bass2jax.partition_id_tensor())
        outs = bass2jax._bass_exec_p.bind(
            *operands,
            out_avals=tuple(out_avals),
            in_names=tuple(all_names),
            out_names=tuple(out_names),
            lowering_input_output_aliases=(),
            sim_require_finite=True,
            sim_require_nnan=True,
            nc=nc,
        )
        return tuple(outs)

    devices = jax.devices()[:n_cores]
    mesh = Mesh(np.asarray(devices), ("core",))
    in_specs = (PartitionSpec("core"),) * (n_params + n_outs)
    out_specs = (PartitionSpec("core"),) * n_outs
    sharded = jax.jit(
        shard_map(_body, mesh=mesh, in_specs=in_specs, out_specs=out_specs,
                  check_rep=False),
        donate_argnums=donate, keep_unused=True,
    )
    sh = NamedSharding(mesh, PartitionSpec("core"))
    concat_in = [
        jax.device_put(
            np.concatenate([np.asarray(in_maps[c][k])
                            for c in range(n_cores)], axis=0), sh)
        for k in in_names
    ]

    def run(n_repeats=1):
        times = []
        outs = None
        for _ in range(n_repeats):
            z = [jax.device_put(np.zeros((n_cores * s0[0], *s0[1:]), d0), sh)
                 for s0, d0 in zero_outs]
            jax.block_until_ready(z)
            t0 = time.perf_counter()
            outs = sharded(*concat_in, *z)
            jax.block_until_ready(outs)
            times.append(time.perf_counter() - t0)
        results = [
            {k: np.asarray(outs[i]).reshape(n_cores, *out_avals[i].shape)[c]
             for i, k in enumerate(out_names)}
            for c in range(n_cores)
        ]
        return results, times

    return run
